# revision 9
# baseline (speedup 1.0000x reference)
"""DiskKinematics histogram-binning kernel for 8x TRN2 NeuronCores.

Strategy (data-parallel over particles, 2M particles/core):
  - Host interleaves pos/vel/mass into one [T, 128, 7G] array per core so
    each super-tile arrives in a single DMA (single completion semaphore —
    TT instructions only have one sync-wait slot).
  - Arithmetic split across ACT (squares/sqrt) and DVE (products, masked
    features), with the radial bin index idx = floor(5*r_cyl) computed via
    the 2^23 magic-number round trick.
  - The 50-bin weighted scatter runs on the TensorEngine: for every group
    of 128 particles, a [128, 50] bf16 one-hot (built by DVE tensor_scalar
    is_equal against a constant iota row) is streamed against the group's
    [128, 7] bf16 feature matrix held as PE weights, accumulating
    hist[7, 50] in f32 PSUM across all groups.
  - Per-core partial histograms [7, 50] are summed and normalized on host.
"""

import numpy as np
import ml_dtypes

import concourse.bass as bass
import concourse.bacc as bacc
import concourse.mybir as mybir
from concourse.tile import TileContext
from concourse.bass_utils import run_bass_kernel_spmd

P = 128
BINS = 50
F = 7
N_CORES = 8
G = 625            # particles per partition per super-tile
B = 25             # matmul groups per one-hot batch tile
W = 7 * G          # row width of combined input

MAGIC_LO = 1.5 * float(2**23) - 0.5
MAGIC_HI = 1.5 * float(2**23)
# multiplier approximating the reference's division by float32(0.2)
C_INV_DR = float(np.float32(1.0 / np.float64(np.float32(0.2))))

f32 = mybir.dt.float32
bf16 = mybir.dt.bfloat16

_CACHE = {}


def _build(n_tiles: int, gpsimd_onehot: bool = False, G: int = G, B: int = B):
    npc = P * G * n_tiles
    W = 7 * G
    nc = bacc.Bacc(None, target_bir_lowering=False, debug=False)
    data = nc.dram_tensor("data", [n_tiles, P, W], f32, kind="ExternalInput")
    iota_in = nc.dram_tensor("iota", [P, BINS], bf16, kind="ExternalInput")
    hist = nc.dram_tensor("hist", [F, BINS], f32, kind="ExternalOutput")

    AO = mybir.AluOpType
    AF = mybir.ActivationFunctionType

    with TileContext(nc) as tc:
        with (
            tc.tile_pool(name="io", bufs=3) as iop,
            tc.tile_pool(name="ar", bufs=2) as arp,
            tc.tile_pool(name="wf", bufs=2) as wfp,
            tc.tile_pool(name="oh", bufs=4) as ohp,
            tc.tile_pool(name="cst", bufs=1) as cst,
            tc.tile_pool(name="ps", bufs=1, space="PSUM") as psp,
        ):
            iota_t = cst.tile([P, BINS], bf16)
            nc.sync.dma_start(out=iota_t[:], in_=iota_in[:])
            ps = psp.tile([F, BINS], f32)
            n_groups_total = n_tiles * G
            gi = 0  # global group counter

            for t in range(n_tiles):
                dat = iop.tile([P, W], f32, tag="dat")
                nc.sync.dma_start(out=dat[:], in_=data[t])

                def pcomp(c):
                    # strided view of interleaved [x y z] positions
                    return dat[:, 0 : 3 * G].rearrange("p (g c) -> p c g", c=3)[
                        :, c : c + 1, :
                    ].rearrange("p c g -> p (c g)")

                def vcomp(c):
                    return dat[:, 3 * G : 6 * G].rearrange(
                        "p (g c) -> p c g", c=3
                    )[:, c : c + 1, :].rearrange("p c g -> p (c g)")

                x = pcomp(0)
                y = pcomp(1)
                vx = vcomp(0)
                vy = vcomp(1)
                vz = vcomp(2)
                m = dat[:, 6 * G : 7 * G]

                xx = arp.tile([P, G], f32, tag="xx")
                yy = arp.tile([P, G], f32, tag="yy")
                rsq = arp.tile([P, G], f32, tag="rsq")
                p25 = arp.tile([P, G], f32, tag="p25")
                t5 = arp.tile([P, G], f32, tag="t5")
                inv_r = arp.tile([P, G], f32, tag="inv_r")
                idxf = arp.tile([P, G], f32, tag="idxf")
                a1 = arp.tile([P, G], f32, tag="a1")
                a2 = arp.tile([P, G], f32, tag="a2")
                b1 = arp.tile([P, G], f32, tag="b1")
                b2 = arp.tile([P, G], f32, tag="b2")
                sdot = arp.tile([P, G], f32, tag="sdot")
                tdot = arp.tile([P, G], f32, tag="tdot")
                vr = arp.tile([P, G], f32, tag="vr")
                vph = arp.tile([P, G], f32, tag="vph")
                wf1f = arp.tile([P, G], f32, tag="wf1f")
                wf3f = arp.tile([P, G], f32, tag="wf3f")
                wf5f = arp.tile([P, G], f32, tag="wf5f")

                # --- binning path (f32): t5 = 5*r_cyl -------------------------
                nc.scalar.activation(xx[:], x, AF.Square, scale=C_INV_DR)
                nc.scalar.activation(yy[:], y, AF.Square, scale=C_INV_DR)
                nc.vector.tensor_tensor(out=rsq[:], in0=xx[:], in1=yy[:], op=AO.add)
                nc.vector.reciprocal(out=p25[:], in_=rsq[:])
                nc.scalar.activation(t5[:], rsq[:], AF.Sqrt)
                nc.scalar.activation(
                    inv_r[:], p25[:], AF.Sqrt, scale=float(C_INV_DR) ** 2
                )
                # idxf = floor(t5): (t5 - 0.5 + 1.5*2^23) RNE-rounds to
                # floor(t5) + 1.5*2^23 (spacing 1.0 in [2^23, 2^24))
                t5h = arp.tile([P, G], f32, tag="t5h")
                nc.vector.tensor_scalar(
                    t5h[:], t5[:], 0.5, 12582912.0, AO.subtract, AO.add
                )
                nc.vector.tensor_scalar(
                    idxf[:], t5h[:], 12582912.0, None, AO.subtract
                )

                # --- velocity projections ------------------------------------
                nc.vector.tensor_tensor(out=a1[:], in0=x, in1=vx, op=AO.mult)
                nc.vector.tensor_tensor(out=a2[:], in0=y, in1=vy, op=AO.mult)
                nc.vector.tensor_tensor(out=b1[:], in0=y, in1=vx, op=AO.mult)
                nc.vector.tensor_tensor(out=b2[:], in0=x, in1=vy, op=AO.mult)
                nc.vector.tensor_tensor(out=sdot[:], in0=a1[:], in1=a2[:], op=AO.add)
                nc.vector.tensor_tensor(
                    out=tdot[:], in0=b1[:], in1=b2[:], op=AO.subtract
                )
                nc.vector.tensor_tensor(out=vr[:], in0=sdot[:], in1=inv_r[:], op=AO.mult)
                nc.vector.tensor_tensor(out=vph[:], in0=tdot[:], in1=inv_r[:], op=AO.mult)

                # --- weighted features (bf16 planar [128, 7*G]) ---------------
                # all writers on DVE so matmuls wait on a single engine sem
                wft = wfp.tile([P, F * G], bf16, tag="wf")

                def wfs(f):
                    return wft[:, f * G : (f + 1) * G]

                nc.vector.tensor_copy(out=wfs(0), in_=m)              # w
                nc.vector.tensor_tensor(out=wf1f[:], in0=m, in1=vr[:], op=AO.mult)
                nc.vector.tensor_copy(out=wfs(1), in_=wf1f[:])        # w*vr
                nc.vector.tensor_tensor(out=wfs(2), in0=wf1f[:], in1=vr[:], op=AO.mult)
                nc.vector.tensor_tensor(out=wf3f[:], in0=m, in1=vph[:], op=AO.mult)
                nc.vector.tensor_copy(out=wfs(3), in_=wf3f[:])        # w*vphi
                nc.vector.tensor_tensor(out=wfs(4), in0=wf3f[:], in1=vph[:], op=AO.mult)
                nc.vector.tensor_tensor(out=wf5f[:], in0=m, in1=vz, op=AO.mult)
                nc.vector.tensor_copy(out=wfs(5), in_=wf5f[:])        # w*vz
                nc.vector.tensor_tensor(out=wfs(6), in0=wf5f[:], in1=vz, op=AO.mult)

                wfv = wft[:].rearrange("p (f g) -> p f g", f=F)

                # --- scatter: one-hot + PE matmul accumulate ------------------
                for b0 in range(0, G, B):
                    oh = ohp.tile([P, B * BINS], bf16, tag="oh")
                    eng = (
                        nc.gpsimd
                        if (gpsimd_onehot and (b0 // B) % 2 == 1)
                        else nc.vector
                    )
                    for j in range(B):
                        g = b0 + j
                        eng.tensor_scalar(
                            oh[:, j * BINS : (j + 1) * BINS],
                            iota_t[:],
                            idxf[:, g : g + 1],
                            None,
                            AO.is_equal,
                        )
                    for j in range(B):
                        g = b0 + j
                        lhsT = wfv[:, :, g : g + 1].rearrange("p f g -> p (f g)")
                        nc.tensor.matmul(
                            out=ps[:],
                            lhsT=lhsT,
                            rhs=oh[:, j * BINS : (j + 1) * BINS],
                            start=(gi == 0),
                            stop=(gi == n_groups_total - 1),
                        )
                        gi += 1

            out_sb = cst.tile([F, BINS], f32)
            nc.vector.tensor_copy(out=out_sb[:], in_=ps[:])
            nc.sync.dma_start(out=hist[:], in_=out_sb[:])

    nc.compile()
    return nc


LAST_RESULTS = None


def _pack_core(positions, velocities, masses, n_tiles):
    """[npc,3]x2 + [npc] -> [T, 128, 7G] interleaved rows."""
    pr = positions.reshape(n_tiles, P, 3 * G)
    vr = velocities.reshape(n_tiles, P, 3 * G)
    mr = masses.reshape(n_tiles, P, G)
    return np.concatenate([pr, vr, mr], axis=2)


def kernel(positions, velocities, masses, trace=False):
    global LAST_RESULTS
    positions = np.ascontiguousarray(np.asarray(positions, dtype=np.float32))
    velocities = np.ascontiguousarray(np.asarray(velocities, dtype=np.float32))
    masses = np.ascontiguousarray(np.asarray(masses, dtype=np.float32))
    n = positions.shape[0]
    assert n % (N_CORES * P * G) == 0, n
    npc = n // N_CORES
    n_tiles = npc // (P * G)

    key = n_tiles
    if key not in _CACHE:
        _CACHE[key] = _build(n_tiles)
    nc = _CACHE[key]

    iota = np.tile(np.arange(BINS, dtype=np.float32), (P, 1)).astype(ml_dtypes.bfloat16)
    in_maps = []
    for k in range(N_CORES):
        sl = slice(k * npc, (k + 1) * npc)
        in_maps.append(
            {
                "data": _pack_core(
                    positions[sl], velocities[sl], masses[sl], n_tiles
                ),
                "iota": iota,
            }
        )

    res = run_bass_kernel_spmd(
        nc, in_maps, core_ids=list(range(N_CORES)), trace=trace
    )
    LAST_RESULTS = res

    hsum = np.zeros((F, BINS), dtype=np.float64)
    for r in res.results:
        hsum += r["hist"].astype(np.float64)

    mass = hsum[0]
    with np.errstate(divide="ignore", invalid="ignore"):
        mm = hsum[1:] / mass
        vr_m, vr2, vph_m, vph2, vz_m, vz2 = (mm[j] for j in range(6))
        vr_sig = np.sqrt(np.maximum(vr2 - vr_m**2, 0.0))
        vph_sig = np.sqrt(np.maximum(vph2 - vph_m**2, 0.0))
        vz_sig = np.sqrt(np.maximum(vz2 - vz_m**2, 0.0))
    kin = np.stack((vph_m, vph_sig, vr_m, vr_sig, vz_m, vz_sig))
    return kin.astype(np.float32)
